# revision 1
# baseline (speedup 1.0000x reference)
"""FourierKAN layer (N=16384, I=128, O=128, G=16) on 8 Trainium2 NeuronCores.

out[n,o] = LN_o( sum_{i,g} cos(g*x[n,i])*Ac[o,i,g] + sin(g*x[n,i])*As[o,i,g]
                 + bias[o] ) * gamma + beta

Strategy (data-parallel over N, 2048 rows/core):
 - Device computes a basis of 33 [I=128, n] fp16 tiles whose span (as
   functions of x) covers all 32 harmonics {cos(gx), sin(gx), g=1..16} + const:
     * direct pairs g in {1,3,8,12}: fp32 range-reduction (rint via +1.5*2^23
       trick) then ACT Sin; cos via Square(Sin(pi*t)) identity.
     * doubling chain 2,4,6,16: Square(c_g) + c_g*s_g.
     * product quads (a,b): 4 elementwise products span cos/sin((a+-b)x).
 - Host solves exact least-squares weights W[b,i,o] (fp64) mapping basis ->
   amplitudes, centers over o (folds LayerNorm mean-subtraction into weights).
 - GEMM: 33 accumulating fp16 matmuls per 512-col tile -> y_c [O,n] PSUM,
   consumed eagerly (all 4 column tiles) as each basis tile is produced.
 - LN: Sigma y_c^2 via ones-matmul; rstd = exp(-0.5*ln(var+eps)) on ACT;
   gamma folded into the rstd broadcast matmul (stationary = gamma row);
   +beta on DVE; LN chains staggered j-major behind the PE tail.
 - Placement: DVE = range reduction + products; ACT = sins + all squares
   (carrier purification folded into Square's free affine); GpSimd = output
   DMA only (Pool-engine elementwise is slow and steals SBUF bandwidth).
 - x is DMAed in two halves so the anchor-1 chain starts early; all input
   DMAs are serialized on the sync queue to prioritize x.
 - Output is produced [O, N]-layout on device; host transposes to (N, O).
"""
import sys

sys.path.insert(0, "/opt/trn_rl_repo")

import numpy as np

import concourse.bass as bass
import concourse.mybir as mybir
from concourse.tile import TileContext
from contextlib import ExitStack

# ---------------------------------------------------------------------------
# walrus in this container accepts at most ONE sync-wait command per
# instruction; TileContext's tail drain and ordinary joins can carry more.
# Patch: split waits onto same-engine InstNoOp carriers.
# ---------------------------------------------------------------------------
import bass_rust
from concourse import tile as _tile


def _patched_drain_and_barrier(self, tick_clock, wait_clock):
    nc = self.nc
    gc = tick_clock.global_clock
    n = len(gc)
    for p in range(n):
        if gc[p] > 0:
            vc = bass_rust.VectorClock([0] * n)
            vc.require_at_least(p, gc[p])
            nop = nc.sync.nop(hint="drain_wait_carrier", nofuse=True)
            wait_clock.add_sem_waits(nop.ins, bass_rust.ScopedClock({None: vc}))
    nc.sync.drain()
    nc.all_engine_barrier()
    assert self.sems is not None
    popped = nc._tile_sem_poison_stack.pop()
    assert popped is self._sem_poison
    nc.clear_and_free_semaphores(list(self.sems.allocated().values()))
    nc.all_engine_barrier()


_orig_lower = _tile.TileContext._lower_ordered_insts


def _patched_lower_ordered_insts(self, ordered):
    for bb_name, insts in ordered.items():
        new = []
        for inst in insts:
            si = getattr(inst, "sync_info", None)
            eng = getattr(inst, "engine", None)
            if (
                si is not None
                and si.on_wait
                and len(si.on_wait) > 1
                and eng is not None
                and isinstance(inst, mybir.Instruction)
            ):
                waits = list(si.on_wait)
                for w in waits[:-1]:
                    new.append(
                        mybir.InstNoOp(
                            name=self.nc.get_next_instruction_name(),
                            sync_info=mybir.SyncInfo(on_wait=[w], on_update=[]),
                            bass_nofuse=True,
                            engine=eng,
                        )
                    )
                inst.sync_info = mybir.SyncInfo(
                    on_wait=[waits[-1]], on_update=list(si.on_update)
                )
            new.append(inst)
        insts[:] = new
    return _orig_lower(self, ordered)


_tile.TileContext._drain_and_barrier = _patched_drain_and_barrier
_tile.TileContext._lower_ordered_insts = _patched_lower_ordered_insts

# ---------------------------------------------------------------------------
# Problem constants
# ---------------------------------------------------------------------------
N, I, O, G = 16384, 128, 128, 16
NCORES = 8
NSH = N // NCORES  # 2048 rows per core
JT = 512  # GEMM moving-tile width
NJ = NSH // JT  # 4
HW = NSH // 2  # half-tile width for pipelined head
F32 = mybir.dt.float32
F16 = mybir.dt.float16
A = mybir.AluOpType
AF = mybir.ActivationFunctionType
TWO_PI = 2.0 * np.pi
RC = 12582912.0  # 1.5 * 2^23 : fp32 round-to-int magic constant
EPS = 1e-5

# ---------------------------------------------------------------------------
# Basis specification. Each op appends basis/mat tiles with an exact harmonic
# expansion dict {('c',g)|('s',g)|('1',0): coef}. Tiles are named; device ops
# are emitted from the same spec so host and device agree exactly.
# ---------------------------------------------------------------------------


def _expand_mul(e1, e2):
    out = {}

    def add(k, v):
        if abs(v) > 1e-15:
            out[k] = out.get(k, 0.0) + v

    for (k1, g1), v1 in e1.items():
        for (k2, g2), v2 in e2.items():
            v = v1 * v2
            if k1 == "1" and k2 == "1":
                add(("1", 0), v)
            elif k1 == "1":
                add((k2, g2), v)
            elif k2 == "1":
                add((k1, g1), v)
            elif k1 == "c" and k2 == "c":
                add(_n("c", g1 + g2), 0.5 * v)
                add(_n("c", g1 - g2), 0.5 * v)
            elif k1 == "s" and k2 == "s":
                add(_n("c", g1 - g2), 0.5 * v)
                add(_n("c", g1 + g2), -0.5 * v)
            elif k1 == "s" and k2 == "c":
                add(_n("s", g1 + g2), 0.5 * v)
                add(_n("s", g1 - g2), 0.5 * v)
            else:  # c * s
                add(_n("s", g1 + g2), 0.5 * v)
                add(_n("s", g1 - g2), -0.5 * v)
    res = {}
    for (k, g), v in out.items():
        if abs(v) > 1e-15:
            res[(k, g)] = res.get((k, g), 0.0) + v
    return {k: v for k, v in res.items() if abs(v) > 1e-15}


def _n(kind, g):
    # canonical harmonic key: cos(-g)=cos(g); sin(-g)=-sin(g); sin(0)=0->dropped
    if g < 0:
        if kind == "c":
            return ("c", -g)
        return ("s_neg", -g)
    if g == 0:
        if kind == "c":
            return ("1", 0)
        return ("zero", 0)
    return (kind, g)


def _expand_mul_fix(e1, e2):
    raw = _expand_mul(e1, e2)
    out = {}
    for (k, g), v in raw.items():
        if k == "s_neg":
            out[("s", g)] = out.get(("s", g), 0.0) - v
        elif k == "zero":
            pass
        else:
            out[(k, g)] = out.get((k, g), 0.0) + v
    return {k: v for k, v in out.items() if abs(v) > 1e-15}


class Spec:
    """Device program spec: named fp16 tiles + exact harmonic expansions."""

    def __init__(self):
        self.exp = {}  # name -> expansion dict
        self.basis = []  # names feeding the GEMM, in order

    def direct(self, g, mat_c=False):
        sn, qn = f"s{g}", f"q{g}"
        self.exp[sn] = {("s", g): 1.0}
        self.exp[qn] = {("1", 0): 0.5, ("c", g): -0.5}
        self.basis += [sn, qn]
        if mat_c:
            self.exp[f"c{g}"] = {("c", g): 1.0}

    def dbl(self, cc, sc, sqn, tn):
        self.exp[sqn] = _expand_mul_fix(self.exp[cc], self.exp[cc])
        self.exp[tn] = _expand_mul_fix(self.exp[cc], self.exp[sc])
        self.basis += [sqn, tn]

    def quad(self, ca, sa, cb, sb):
        names = []
        for (x, y) in ((ca, cb), (sa, sb), (sa, cb), (ca, sb)):
            pn = f"p_{x}_{y}"
            self.exp[pn] = _expand_mul_fix(self.exp[x], self.exp[y])
            self.basis.append(pn)
            names.append(pn)
        return names


def build_spec():
    sp = Spec()
    sp.direct(1, mat_c=True)
    sp.direct(3, mat_c=True)
    sp.direct(8, mat_c=True)
    sp.direct(12)
    # sq tiles are exact c_g^2 via ACT Square(-2*q_g+1); t/product tiles
    # use the raw q carriers (pinv handles the affine mixing).
    sp.exp["sq2"] = _expand_mul_fix(sp.exp["c1"], sp.exp["c1"])
    sp.exp["t2"] = _expand_mul_fix(sp.exp["q1"], sp.exp["s1"])
    sp.basis += ["sq2", "t2"]
    sp.dbl("sq2", "t2", "sq4", "t4")     # 4
    sp.exp["sq6"] = _expand_mul_fix(sp.exp["c3"], sp.exp["c3"])
    sp.exp["t6"] = _expand_mul_fix(sp.exp["q3"], sp.exp["s3"])
    sp.basis += ["sq6", "t6"]
    sp.exp["sq16"] = _expand_mul_fix(sp.exp["c8"], sp.exp["c8"])
    sp.exp["t16"] = _expand_mul_fix(sp.exp["q8"], sp.exp["s8"])
    sp.basis += ["sq16", "t16"]
    sp.quad("sq6", "t6", "q1", "s1")     # {7,5}
    sp.quad("q12", "s12", "q1", "s1")    # {13,11}
    sp.quad("q12", "s12", "sq2", "t2")   # {14,10}
    sp.quad("q12", "s12", "q3", "s3")    # {15,9}
    sp.exp["one"] = {("1", 0): 1.0}
    sp.basis.insert(0, "one")
    return sp


SPEC = build_spec()
B = len(SPEC.basis)  # 37

HARMONICS = [("1", 0)] + [("c", g) for g in range(1, G + 1)] + [
    ("s", g) for g in range(1, G + 1)
]  # 33


def solve_weights(cos_amp, sin_amp, bias):
    """W[b, i, o] fp64 -> fp16, LN-mean-centered over o."""
    M = np.zeros((B, len(HARMONICS)))
    hidx = {h: k for k, h in enumerate(HARMONICS)}
    for bi, name in enumerate(SPEC.basis):
        for h, v in SPEC.exp[name].items():
            M[bi, hidx[h]] = v
    T = np.zeros((len(HARMONICS), I, O))
    T[0] = bias[None, :] / I
    for g in range(1, G + 1):
        T[hidx[("c", g)]] = cos_amp[:, :, g - 1].T  # [i, o]
        T[hidx[("s", g)]] = sin_amp[:, :, g - 1].T
    piv = np.linalg.pinv(M.T)  # [B, 33]
    resid = np.abs(M.T @ piv - np.eye(len(HARMONICS))).max()
    assert resid < 1e-9, f"basis does not span harmonics: resid={resid}"
    W = np.einsum("bh,hio->bio", piv, T)
    W = W - W.mean(axis=2, keepdims=True)  # center over o (LN mean fold)
    return W


# ---------------------------------------------------------------------------
# Device program
# ---------------------------------------------------------------------------


def build_device_program():
    nc = bass.Bass()
    x0_in = nc.declare_dram_parameter("x0_sh", [I, HW], F32, isOutput=False)
    x1_in = nc.declare_dram_parameter("x1_sh", [I, HW], F32, isOutput=False)
    w_in = nc.declare_dram_parameter("w_all", [I, B * O], F16, isOutput=False)
    g_in = nc.declare_dram_parameter("gam_row", [1, O], F16, isOutput=False)
    b_in = nc.declare_dram_parameter("bet", [O, 1], F32, isOutput=False)
    out_d = nc.declare_dram_parameter("out_sh", [O, NSH], F32, isOutput=True)

    with ExitStack() as ctx:
        tc = ctx.enter_context(TileContext(nc))
        pool = ctx.enter_context(tc.tile_pool(name="main", bufs=1))
        scr = ctx.enter_context(tc.tile_pool(name="scratch", bufs=2))
        pj = ctx.enter_context(tc.tile_pool(name="psy", bufs=1, space="PSUM"))
        pv = ctx.enter_context(tc.tile_pool(name="psv", bufs=2, space="PSUM"))
        pb = ctx.enter_context(tc.tile_pool(name="psb", bufs=2, space="PSUM"))

        # ---- input DMAs: x halves + small params on sync queue; weights on
        # gpsimd queue so both streams overlap.
        xh = [
            pool.tile([I, HW], F32, tag="x0", name="x0"),
            pool.tile([I, HW], F32, tag="x1", name="x1"),
        ]
        nc.sync.dma_start(out=xh[0][:], in_=x0_in[:])
        nc.sync.dma_start(out=xh[1][:], in_=x1_in[:])
        wts = pool.tile([I, B * O], F16, tag="wts", name="wts")
        nc.sync.dma_start(out=wts[:], in_=w_in[:])
        gam = pool.tile([1, O], F16, tag="gam", name="gam")
        bet = pool.tile([O, 1], F32, tag="bet", name="bet")
        nc.sync.dma_start(out=gam[:], in_=g_in[:])
        nc.sync.dma_start(out=bet[:], in_=b_in[:])

        eps_t = pool.tile([1, 1], F32, tag="eps", name="eps")
        nc.vector.memset(eps_t[:], EPS)
        ones_col = pool.tile([I, 1], F16, tag="ones_col", name="ones_col")
        nc.vector.memset(ones_col[:], 1.0)
        # ACT table prewarm: force the Sin table set load before t1 is ready.
        pw = pool.tile([1, 1], F16, tag="pw", name="pw")
        nc.scalar.activation(pw[:], eps_t[:], AF.Sin)

        HS = [slice(0, HW), slice(HW, NSH)]  # column halves

        tiles = {}

        def t16f(name):
            if name.startswith("sh"):
                # half-angle sin scratch: short-lived, rotate 2 buffers
                t = scr.tile([I, NSH], F16, tag="sh", name="b_" + name)
            else:
                t = pool.tile([I, NSH], F16, tag="b_" + name,
                              name="b_" + name)
            tiles[name] = t
            return t

        # --- MM bookkeeping: eager-j GEMM --------------------------------
        ys = [pj.tile([O, JT], F32, tag=f"y{j}", name=f"y{j}") for j in
              range(NJ)]
        ones_bc = ones_col[:].to_broadcast((I, JT))
        n_mm = [0] * NJ  # matmuls issued per j
        bidx = {name: i for i, name in enumerate(SPEC.basis)}

        def emit_mm(name, js):
            bi = bidx[name]
            for j in js:
                rhs = (
                    ones_bc
                    if name == "one"
                    else tiles[name][:, j * JT: (j + 1) * JT]
                )
                nc.tensor.matmul(
                    ys[j][:],
                    wts[:, bi * O: (bi + 1) * O],
                    rhs,
                    start=(n_mm[j] == 0),
                    stop=(n_mm[j] == B - 1),
                )
                n_mm[j] += 1

        # --- frac helpers -------------------------------------------------
        # w and t scratch tiles rotate through 2 shared buffers each (the
        # per-anchor lifetimes are short and mostly disjoint).
        fw = {}

        def frac_vw(g, h):
            """w = rint(x*s) for anchor g on column-half h (DVE, in-place)."""
            s = float(np.float32(g / TWO_PI))
            if g not in fw:
                fw[g] = scr.tile([I, NSH], F32, tag="fw", name=f"fw{g}")
            for hh in ((0, 1) if h is None else (h,)):
                hs = HS[hh]
                nc.vector.tensor_scalar(
                    fw[g][:, hs], xh[hh][:], s, RC, A.mult, A.add
                )
                nc.vector.tensor_scalar(
                    fw[g][:, hs], fw[g][:, hs], RC, None, A.subtract
                )

        def frac_t(g, h, eng):
            """t = x*s - w on half h (STT on chosen engine)."""
            s = float(np.float32(g / TWO_PI))
            if f"t{g}" not in tiles:
                tiles[f"t{g}"] = scr.tile([I, NSH], F32, tag="ft",
                                          name=f"ft{g}")
            for hh in ((0, 1) if h is None else (h,)):
                hs = HS[hh]
                eng.scalar_tensor_tensor(
                    tiles[f"t{g}"][:, hs], xh[hh][:], s, fw[g][:, hs],
                    A.mult, A.subtract,
                )

        def sin_op(g, name, scale, h=None):
            if name not in tiles:
                t16f(name)
            src = tiles[f"t{g}"]
            if h is None:
                nc.scalar.activation(tiles[name][:], src[:], AF.Sin,
                                     scale=scale)
            else:
                hs = HS[h]
                nc.scalar.activation(tiles[name][:, hs], src[:, hs], AF.Sin,
                                     scale=scale)

        def sq_act(src, dst, scale=1.0, bias=None, h=None):
            if dst not in tiles:
                t16f(dst)
            kw = {"scale": scale}
            if bias is not None:
                kw["bias"] = bias
            if h is None:
                nc.scalar.activation(tiles[dst][:], tiles[src][:], AF.Square,
                                     **kw)
            else:
                hs = HS[h]
                nc.scalar.activation(tiles[dst][:, hs], tiles[src][:, hs],
                                     AF.Square, **kw)

        def mul_dve(a, b, dst):
            nc.vector.tensor_tensor(t16f(dst)[:], tiles[a][:], tiles[b][:],
                                    A.mult)

        def mul_gps(a, b, dst):
            nc.gpsimd.tensor_tensor(t16f(dst)[:], tiles[a][:], tiles[b][:],
                                    A.mult)

        def ts_dve(src, dst, mul, add):
            nc.vector.tensor_scalar(
                t16f(dst)[:], tiles[src][:], mul, add, A.mult, A.add
            )

        # --- stats / finalize --------------------------------------------
        rstds = {}

        def emit_stats_pre(j):
            sq = scr.tile([O, JT], F16, tag="sq", name="sq", bufs=1)
            nc.scalar.activation(sq[:], ys[j][:], AF.Square)
            vps = pv.tile([1, JT], F32, tag="vps", name="vps")
            nc.tensor.matmul(vps[:], ones_col[:], sq[:], start=True,
                             stop=True)
            lv = scr.tile([1, JT], F32, tag="lv", name="lv", bufs=1)
            nc.scalar.activation(
                lv[:], vps[:], AF.Ln, scale=1.0 / O, bias=eps_t[:]
            )
            var_j = scr.tile([1, JT], F16, tag="var_j", name="var_j")
            nc.scalar.activation(var_j[:], lv[:], AF.Exp, scale=-0.5)
            rstds[j] = var_j

        def emit_finalize(j):
            bc = pb.tile([O, JT], F32, tag="bc", name="bc")
            nc.tensor.matmul(bc[:], gam[:], rstds[j][:], start=True,
                             stop=True)
            rb = scr.tile([O, JT], F16, tag="rb", name="rb")
            nc.scalar.activation(rb[:], bc[:], AF.Copy)
            oj = scr.tile([O, JT], F32, tag="oj", name="oj")
            nc.vector.tensor_tensor(oj[:], ys[j][:], rb[:], A.mult)
            nc.vector.tensor_scalar(oj[:], oj[:], bet[:], None, A.add)
            nc.gpsimd.dma_start(out=out_d[:, j * JT: (j + 1) * JT],
                                in_=oj[:])

        # =================================================================
        # Emission. Program order per engine == queue order; ops are laid
        # out in dependency waves so no engine stalls behind a not-ready op.
        # =================================================================

        # -- fracs: anchor 1 at half granularity (unblocks ACT early);
        # anchors 12/3/8 full-tile, all on DVE.
        frac_vw(1, 0)
        frac_t(1, 0, nc.vector)
        frac_vw(1, 1)
        frac_t(1, 1, nc.vector)
        for g in (12, 3, 8):
            frac_vw(g, None)
            frac_t(g, None, nc.vector)

        # -- ACT spine: sins + all squares (affine-folded purification).
        sin_op(1, "s1", TWO_PI, h=0)
        sin_op(1, "sh1", np.pi, h=0)
        sq_act("sh1", "q1", h=0)
        sin_op(1, "s1", TWO_PI, h=1)
        sin_op(1, "sh1", np.pi, h=1)
        sq_act("sh1", "q1", h=1)
        emit_mm("one", range(NJ))
        emit_mm("s1", (0, 1))
        emit_mm("q1", (0, 1))
        emit_mm("s1", (2, 3))
        emit_mm("q1", (2, 3))

        sin_op(12, "s12", TWO_PI)
        sin_op(12, "sh12", np.pi)
        sq_act("sh12", "q12")
        emit_mm("s12", range(NJ))
        emit_mm("q12", range(NJ))

        # DVE products of the 1-family; ACT continues
        mul_dve("q1", "s1", "t2")
        emit_mm("t2", range(NJ))
        mul_dve("q12", "q1", "p_q12_q1")
        mul_dve("s12", "s1", "p_s12_s1")
        mul_dve("s12", "q1", "p_s12_q1")
        mul_dve("q12", "s1", "p_q12_s1")
        emit_mm("p_q12_q1", range(NJ))
        emit_mm("p_s12_s1", range(NJ))
        emit_mm("p_s12_q1", range(NJ))
        emit_mm("p_q12_s1", range(NJ))

        sin_op(3, "s3", TWO_PI)
        sin_op(3, "sh3", np.pi)
        sq_act("sh3", "q3")
        sq_act("q1", "sq2", scale=-2.0, bias=1.0)
        emit_mm("s3", range(NJ))
        emit_mm("q3", range(NJ))
        emit_mm("sq2", range(NJ))

        # DVE: (12,2) quads + t4 as sq2 lands
        mul_dve("q12", "sq2", "p_q12_sq2")
        mul_dve("s12", "t2", "p_s12_t2")
        mul_dve("s12", "sq2", "p_s12_sq2")
        mul_dve("q12", "t2", "p_q12_t2")
        mul_dve("sq2", "t2", "t4")
        emit_mm("p_q12_sq2", range(NJ))
        emit_mm("p_s12_t2", range(NJ))
        emit_mm("p_s12_sq2", range(NJ))
        emit_mm("p_q12_t2", range(NJ))
        emit_mm("t4", range(NJ))

        sin_op(8, "s8", TWO_PI)
        sin_op(8, "sh8", np.pi)
        sq_act("sh8", "q8")
        sq_act("q3", "sq6", scale=-2.0, bias=1.0)
        sq_act("sq2", "sq4")
        emit_mm("s8", range(NJ))
        emit_mm("q8", range(NJ))

        # DVE: 3-family products
        mul_dve("q3", "s3", "t6")
        emit_mm("t6", range(NJ))
        mul_dve("q12", "q3", "p_q12_q3")
        mul_dve("s12", "s3", "p_s12_s3")
        mul_dve("s12", "q3", "p_s12_q3")
        mul_dve("q12", "s3", "p_q12_s3")
        emit_mm("p_q12_q3", range(NJ))
        emit_mm("p_s12_s3", range(NJ))
        emit_mm("p_s12_q3", range(NJ))
        emit_mm("p_q12_s3", range(NJ))
        emit_mm("sq6", range(NJ))
        emit_mm("sq4", range(NJ))

        # ACT: sq16; DVE: 6- and 16-family
        sq_act("q8", "sq16", scale=-2.0, bias=1.0)
        mul_dve("sq6", "q1", "p_sq6_q1")
        mul_dve("t6", "s1", "p_t6_s1")
        mul_dve("t6", "q1", "p_t6_q1")
        mul_dve("sq6", "s1", "p_sq6_s1")
        mul_dve("q8", "s8", "t16")
        # j-major tail: j0 finishes first so the LN chains stagger behind
        # the PE's remaining j1-j3 streams.
        tail = ["p_sq6_q1", "p_t6_s1", "p_t6_q1", "p_sq6_s1", "sq16", "t16"]
        for name in tail:
            emit_mm(name, (0,))
        emit_stats_pre(0)
        for name in tail:
            emit_mm(name, (1,))
        emit_stats_pre(1)
        emit_finalize(0)
        for name in tail:
            emit_mm(name, (2,))
        emit_stats_pre(2)
        emit_finalize(1)
        for name in tail:
            emit_mm(name, (3,))
        emit_stats_pre(3)
        emit_finalize(2)
        emit_finalize(3)
        assert n_mm == [B] * NJ, n_mm
    return nc


_NC_CACHE = None


def kernel(x, cos_amplitudes, sin_amplitudes, bias, ln_gamma, ln_beta):
    global _NC_CACHE
    from concourse.bass_utils import run_bass_kernel_spmd

    x = np.asarray(x, dtype=np.float32)
    ca = np.asarray(cos_amplitudes, dtype=np.float64)
    sa = np.asarray(sin_amplitudes, dtype=np.float64)
    bv = np.asarray(bias, dtype=np.float64)
    gv = np.asarray(ln_gamma, dtype=np.float16).reshape(1, O)
    be = np.asarray(ln_beta, dtype=np.float32).reshape(O, 1)

    W = solve_weights(ca, sa, bv)  # [B, I, O] fp64 centered
    w_all = np.ascontiguousarray(
        W.transpose(1, 0, 2).reshape(I, B * O)
    ).astype(np.float16)

    xT = np.ascontiguousarray(x.T)  # [I, N]

    if _NC_CACHE is None:
        _NC_CACHE = build_device_program()
    nc = _NC_CACHE

    in_maps = []
    for c in range(NCORES):
        xs = xT[:, c * NSH: (c + 1) * NSH]
        in_maps.append(
            {
                "x0_sh": np.ascontiguousarray(xs[:, :HW]),
                "x1_sh": np.ascontiguousarray(xs[:, HW:]),
                "w_all": w_all,
                "gam_row": gv,
                "bet": be,
            }
        )
    res = run_bass_kernel_spmd(nc, in_maps, list(range(NCORES)))
    outs = [res.results[c]["out_sh"] for c in range(NCORES)]
    full = np.concatenate(outs, axis=1)  # [O, N]
    return np.ascontiguousarray(full.T).astype(np.float32)

